# revision 1
# baseline (speedup 1.0000x reference)
"""Trainium2 Bass kernel for geodesic convolution (gnn_message_passing).

Computation (per vertex v):
  x[v,i,j,c]  = sum_t bary_w[v,i,j,t] * signal[bary_idx[v,i,j,t], c]
  conv[v,k,d] = sum_{i,j,c} x[v,i,j,c] * K[i,(j+k)%NT,c,d]
  out[v,:]    = relu(conv[v, argmax_k ||conv[v,k,:]||, :])

Strategy: shard V across 8 cores (data-parallel). Per core, per tile of 128
vertices: dma_gather of the 3*5*8 = 120 signal rows per vertex (v-major
layout; int16 indices biased by -32768 against a mid-tensor base so the
signed offsets cover all 50000 rows), DVE weighted sum over the 3 barycentric
taps, PE transpose of x to channel-major, one accumulated matmul chain
against the pre-rotated kernel matrix W[(i,j,c),(k,d)], then
norms/argmax/select/relu epilogue on DVE.
"""

import numpy as np

# Problem constants (hardcoded; kernel.py must be self-contained).
V, NR, NT, CIN, COUT = 50000, 5, 8, 64, 64
NCORES = 8
VPC = V // NCORES            # 6250 vertices per core
TPT = 128                    # vertices per tile (partition dim)
NTILES = -(-VPC // TPT)      # 49
VPAD = NTILES * TPT          # 6272
IJ = NR * NT                 # 40
E = IJ * 3                   # 120 gathered rows per vertex
EP = E + 1                   # +1 pad slot per partition (trailing-trim guard)
NIDX = EP * TPT              # 15488 gather indices per tile
NS = NIDX // 16              # idx free dim in wrapped-16 layout
KC = IJ * CIN                # 2560 contraction dim
NCHUNK = KC // 128           # 20
ND = NT * COUT               # 512 output cols (k,d)

_CACHE = {}


def build_program(ntiles=NTILES, v_src=V, repeat=1):
    """Build the Bacc program for one SPMD core. Returns compiled nc.

    repeat > 1 duplicates the whole tile loop (same inputs/outputs) for
    wall-clock slope timing; the extra passes just overwrite the outputs.
    """
    import concourse.bass as bass
    import concourse.mybir as mybir
    import concourse.tile as tile
    from concourse import bacc
    from concourse.masks import make_identity

    f32 = mybir.dt.float32
    i16 = mybir.dt.int16

    base = 32768 if v_src > 32768 else 0

    nc = bacc.Bacc(
        "TRN2",
        target_bir_lowering=False,
        debug=False,
        enable_asserts=False,
        num_devices=NCORES,
    )
    vpad = ntiles * TPT
    sig_d = nc.dram_tensor("signal", [v_src, CIN], f32, kind="ExternalInput")
    wv_d = nc.dram_tensor("wv", [vpad, E], f32, kind="ExternalInput")
    idx_d = nc.dram_tensor("idx16", [ntiles * 128, NS], i16, kind="ExternalInput")
    wm_d = nc.dram_tensor("wm", [KC, ND], f32, kind="ExternalInput")
    out_d = nc.dram_tensor("out", [vpad, COUT], f32, kind="ExternalOutput")

    sig_base = sig_d.ap()[base:, :] if base else sig_d.ap()

    with tile.TileContext(nc) as tc:
        with (
            tc.tile_pool(name="const", bufs=1) as cpool,
            tc.tile_pool(name="io", bufs=2) as iopool,
            tc.tile_pool(name="g", bufs=3) as gpool,
            tc.tile_pool(name="x", bufs=2) as xpool,
            tc.tile_pool(name="xT", bufs=3) as xtpool,
            tc.tile_pool(name="epi", bufs=2) as epool,
            tc.tile_pool(name="psA", bufs=2, space="PSUM") as psA,
            tc.tile_pool(name="psB", bufs=3, space="PSUM") as psB,
        ):
            # Resident: rotated kernel matrix [128, NCHUNK, 512] (chunk k of
            # contraction rows at [:, k, :]) and transpose identity.
            wm_t = cpool.tile([128, NCHUNK, ND], f32)
            nc.sync.dma_start(
                out=wm_t[:],
                in_=wm_d.ap().rearrange("(k p) n -> p k n", p=128),
            )
            ident = cpool.tile([128, 128], f32)
            make_identity(nc, ident[:])

            for it_rep in range(ntiles * repeat):
                it = it_rep % ntiles
                rows = slice(it * TPT, (it + 1) * TPT)
                w_t = iopool.tile([128, E], f32, tag="w")
                i_t = iopool.tile([128, NS], i16, tag="i")
                nc.sync.dma_start(out=w_t[:], in_=wv_d.ap()[rows, :])
                nc.sync.dma_start(out=i_t[:], in_=idx_d.ap()[rows, :])

                # Gather: g[p, e, :] = signal[idx[v_p, e], :]  (e < E; slot E is pad)
                g_t = gpool.tile([128, EP, CIN], f32)
                nc.gpsimd.dma_gather(
                    out_ap=g_t[:], in_ap=sig_base, idxs_ap=i_t[:],
                    num_idxs=NIDX, num_idxs_reg=NIDX, elem_size=CIN,
                    single_packet=False,
                )

                # Weighted sum over the 3 barycentric taps.
                g_e = g_t[:, :E, :]
                nc.vector.tensor_tensor(
                    out=g_e,
                    in0=g_e,
                    in1=w_t[:].unsqueeze(-1).to_broadcast([128, E, CIN]),
                    op=mybir.AluOpType.mult,
                )
                g4 = g_e.rearrange("p (ij t) c -> p ij t c", t=3)
                x_t = xpool.tile([128, IJ, CIN], f32)
                nc.vector.tensor_tensor(
                    out=x_t[:], in0=g4[:, :, 0, :], in1=g4[:, :, 1, :],
                    op=mybir.AluOpType.add,
                )
                nc.vector.tensor_tensor(
                    out=x_t[:], in0=x_t[:], in1=g4[:, :, 2, :],
                    op=mybir.AluOpType.add,
                )

                # Transpose x to channel-major and run the matmul chain.
                x2 = x_t[:].rearrange("p ij c -> p (ij c)")
                conv_p = psA.tile([128, ND], f32, tag="conv")
                for k in range(NCHUNK):
                    pt = psB.tile([128, 128], f32, tag="pt")
                    nc.tensor.transpose(
                        pt[:], x2[:, k * 128:(k + 1) * 128], ident[:]
                    )
                    xT = xtpool.tile([128, 128], f32, tag="xT")
                    nc.scalar.copy(out=xT[:], in_=pt[:])
                    nc.tensor.matmul(
                        conv_p[:],
                        lhsT=xT[:],
                        rhs=wm_t[:, k, :],
                        start=(k == 0),
                        stop=(k == NCHUNK - 1),
                    )

                # Epilogue: norms over d, argmax over k (via is_equal mask),
                # masked-sum select, relu.
                sq_t = epool.tile([128, ND], f32, tag="sq")
                nc.scalar.activation(
                    out=sq_t[:], in_=conv_p[:],
                    func=mybir.ActivationFunctionType.Square,
                )
                norm_t = epool.tile([128, NT], f32, tag="norm")
                nc.vector.tensor_reduce(
                    out=norm_t[:],
                    in_=sq_t[:].rearrange("p (k d) -> p k d", d=COUT),
                    axis=mybir.AxisListType.X,
                    op=mybir.AluOpType.add,
                )
                mx_t = epool.tile([128, 1], f32, tag="mx")
                nc.vector.tensor_reduce(
                    out=mx_t[:], in_=norm_t[:],
                    axis=mybir.AxisListType.X, op=mybir.AluOpType.max,
                )
                mask_t = epool.tile([128, NT], f32, tag="mask")
                nc.vector.tensor_scalar(
                    out=mask_t[:], in0=norm_t[:], scalar1=mx_t[:], scalar2=None,
                    op0=mybir.AluOpType.is_equal,
                )
                msel_t = epool.tile([128, NT, COUT], f32, tag="msel")
                nc.vector.tensor_tensor(
                    out=msel_t[:],
                    in0=conv_p[:].rearrange("p (k d) -> p k d", d=COUT),
                    in1=mask_t[:].unsqueeze(-1).to_broadcast([128, NT, COUT]),
                    op=mybir.AluOpType.mult,
                )
                o_t = epool.tile([128, COUT], f32, tag="o")
                nc.vector.tensor_reduce(
                    out=o_t[:],
                    in_=msel_t[:].rearrange("p k d -> p d k"),
                    axis=mybir.AxisListType.X,
                    op=mybir.AluOpType.add,
                )
                nc.vector.tensor_scalar_max(o_t[:], o_t[:], 0.0)
                nc.sync.dma_start(out=out_d.ap()[rows, :], in_=o_t[:])

    nc.compile()
    return nc


def make_idx16(idx_vp, ntiles, base):
    """[vpad, E] int32 row indices -> [ntiles*128, NS] wrapped int16.

    Gather list position n = e*128 + p must hold idx[tile*128 + p, e]; the
    ucode reads logical position i from wrapped[i % 16, i // 16], replicated
    across the 8 Q7 cores (16 partitions each). Slot E is a pad row of
    positive indices so the ucode's trailing-negative trim never fires.
    """
    vpad = ntiles * TPT
    out = np.empty((ntiles, 128, NS), np.int16)
    for t in range(ntiles):
        blk = idx_vp[t * TPT:(t + 1) * TPT]          # [128, E]
        lst = np.full(NIDX, 1, np.int32)             # pad slots -> row base+1
        lst[: E * 128] = (blk.T.astype(np.int32) - base).ravel()  # n = e*128+p
        w = lst.reshape(-1, 16).T                     # [16, NIDX/16]
        out[t] = np.tile(w, (8, 1))
    return out.reshape(ntiles * 128, NS)


def _host_prep(signal, bary_w, bary_idx, kernel):
    """Build per-core input maps. All host-side numpy, not timed."""
    jj = np.arange(NT)
    rot = kernel[:, (jj[:, None] + jj[None, :]) % NT, :, :]  # [i,j,k,c,d]
    wm = np.ascontiguousarray(
        rot.transpose(0, 1, 3, 2, 4).reshape(KC, ND), dtype=np.float32
    )
    sig = np.ascontiguousarray(signal, dtype=np.float32)
    base = 32768
    wv_full = bary_w.reshape(V, E).astype(np.float32)
    idx_full = bary_idx.reshape(V, E).astype(np.int32)
    in_maps = []
    for c in range(NCORES):
        sl = slice(c * VPC, (c + 1) * VPC)
        wv = np.zeros((VPAD, E), np.float32)
        wv[:VPC] = wv_full[sl]
        idx = np.zeros((VPAD, E), np.int32)
        idx[:VPC] = idx_full[sl]
        in_maps.append({
            "signal": sig,
            "wv": wv,
            "idx16": make_idx16(idx, NTILES, base),
            "wm": wm,
        })
    return in_maps


def kernel(signal, bary_w, bary_idx, kernel):
    from concourse.bass_utils import run_bass_kernel_spmd

    if "nc" not in _CACHE:
        _CACHE["nc"] = build_program()
    nc = _CACHE["nc"]
    in_maps = _host_prep(signal, bary_w, bary_idx, kernel)
    res = run_bass_kernel_spmd(nc, in_maps, core_ids=list(range(NCORES)))
    out = np.concatenate(
        [res.results[c]["out"][:VPC] for c in range(NCORES)], axis=0
    )
    return out.astype(np.float32)



# revision 2
# speedup vs baseline: 21.5518x; 21.5518x over previous
"""Trainium2 Bass kernel for geodesic convolution (gnn_message_passing).

Computation (per vertex v):
  x[v,i,j,c]  = sum_t bary_w[v,i,j,t] * signal[bary_idx[v,i,j,t], c]
  conv[v,k,d] = sum_{i,j,c} x[v,i,j,c] * K[i,(j+k)%NT,c,d]
  out[v,:]    = relu(conv[v, argmax_k ||conv[v,k,:]||, :])

Strategy: shard V across 8 cores (data-parallel). The whole tile loop runs
inside tc.For_i hardware loops (static program size is independent of the
tile count and of the timing repeat factor). Per 128-vertex tile:
HBM dma_gather of the 120 signal rows per vertex (int16 indices biased
against a mid-tensor base), DVE weighted tap-sum, PE transpose to
channel-major, a 20-chunk f32 matmul accumulation against the pre-rotated
kernel matrix, then norms/argmax/select/relu epilogue. The gather for tile
t+1 is issued before the compute of tile t (two gather buffers) so the DMA
gather overlaps the PE/DVE compute.
"""

import numpy as np

# Problem constants (hardcoded; kernel.py must be self-contained).
V, NR, NT, CIN, COUT = 50000, 5, 8, 64, 64
NCORES = 8
VPC = V // NCORES            # 6250 vertices per core
TPT = 128                    # vertices per tile (partition dim)
NTILES = -(-VPC // TPT)      # 49
VPAD = NTILES * TPT          # 6272
IJ = NR * NT                 # 40
E = IJ * 3                   # 120 gathered rows per vertex
EP = E + 1                   # +1 pad slot (trailing-trim guard)
NIDX = EP * TPT              # 15488 gather indices per tile
NS = NIDX // 16              # idx free dim in wrapped-16 layout
KC = IJ * CIN                # 2560 contraction dim
NCHUNK = KC // 128           # 20
ND = NT * COUT               # 512 output cols (k,d)
NTPAD = NTILES + 1           # one extra tile of idx/wv padding for prefetch

_CACHE = {}


def build_program(repeat=1):
    """Build the Bacc program for one SPMD core. Returns compiled nc.

    repeat is a For_i loop bound around the tile loop: the static program
    is identical for any repeat, so wall-clock slope between repeat values
    measures pure per-pass hardware time.
    """
    import concourse.bass as bass
    import concourse.mybir as mybir
    import concourse.tile as tile
    from concourse import bacc
    from concourse.bass import ts
    from concourse.masks import make_identity

    f32 = mybir.dt.float32
    i16 = mybir.dt.int16

    nc = bacc.Bacc(
        "TRN2",
        target_bir_lowering=False,
        debug=False,
        enable_asserts=False,
        num_devices=NCORES,
    )
    sig_d = nc.dram_tensor("signal", [V, CIN], f32, kind="ExternalInput")
    wv_d = nc.dram_tensor("wv", [NTPAD * TPT, E], f32, kind="ExternalInput")
    idx_d = nc.dram_tensor("idx16", [NTPAD * TPT, NS], i16,
                           kind="ExternalInput")
    wm_d = nc.dram_tensor("wm", [KC, ND], f32, kind="ExternalInput")
    out_d = nc.dram_tensor("out", [VPAD, COUT], f32, kind="ExternalOutput")
    sig_base = sig_d.ap()[32768:, :]

    with tile.TileContext(nc) as tc:
        with (
            tc.tile_pool(name="const", bufs=1) as cpool,
            tc.tile_pool(name="io", bufs=1) as iopool,
            tc.tile_pool(name="g", bufs=1) as gpool,
            tc.tile_pool(name="x", bufs=1) as xpool,
            tc.tile_pool(name="xT", bufs=2) as xtpool,
            tc.tile_pool(name="epi", bufs=1) as epool,
            tc.tile_pool(name="psA", bufs=2, space="PSUM") as psA,
            tc.tile_pool(name="psB", bufs=2, space="PSUM") as psB,
        ):
            # Residents: rotated kernel matrix and transpose identity.
            wm_t = cpool.tile([128, NCHUNK, ND], f32)
            nc.sync.dma_start(
                out=wm_t[:],
                in_=wm_d.ap().rearrange("(k p) n -> p k n", p=128),
            )
            ident = cpool.tile([128, 128], f32)
            make_identity(nc, ident[:])

            # Two gather buffers for cross-tile overlap.
            g0 = gpool.tile([128, EP, CIN], f32, tag="g0")
            g1 = gpool.tile([128, EP, CIN], f32, tag="g1")
            i0 = iopool.tile([128, NS], i16, tag="i0")
            i1 = iopool.tile([128, NS], i16, tag="i1")
            w_t = iopool.tile([128, E], f32, tag="w")
            x_t = xpool.tile([128, IJ, CIN], f32)
            sq_t = epool.tile([128, ND], f32, tag="sq")
            norm_t = epool.tile([128, NT], f32, tag="norm")
            mx_t = epool.tile([128, 1], f32, tag="mx")
            mask_t = epool.tile([128, NT], f32, tag="mask")
            msel_t = epool.tile([128, NT, COUT], f32, tag="msel")
            o_t = epool.tile([128, COUT], f32, tag="o")

            def gather(i_t, g_t, tidx):
                nc.sync.dma_start(out=i_t[:], in_=idx_d.ap()[ts(tidx, TPT), :])
                nc.gpsimd.dma_gather(
                    out_ap=g_t[:], in_ap=sig_base, idxs_ap=i_t[:],
                    num_idxs=NIDX, num_idxs_reg=NIDX, elem_size=CIN,
                    single_packet=False,
                )

            def compute(g_t, tidx):
                nc.sync.dma_start(out=w_t[:], in_=wv_d.ap()[ts(tidx, TPT), :])
                g_e = g_t[:, :E, :]
                nc.vector.tensor_tensor(
                    out=g_e, in0=g_e,
                    in1=w_t[:].unsqueeze(-1).to_broadcast([128, E, CIN]),
                    op=mybir.AluOpType.mult,
                )
                g4 = g_e.rearrange("p (ij t) c -> p ij t c", t=3)
                nc.vector.tensor_tensor(
                    out=x_t[:], in0=g4[:, :, 0, :], in1=g4[:, :, 1, :],
                    op=mybir.AluOpType.add,
                )
                nc.vector.tensor_tensor(
                    out=x_t[:], in0=x_t[:], in1=g4[:, :, 2, :],
                    op=mybir.AluOpType.add,
                )
                x2 = x_t[:].rearrange("p ij c -> p (ij c)")
                conv_p = psA.tile([128, ND], f32, tag="conv")
                for k in range(NCHUNK):
                    pt = psB.tile([128, 128], f32, tag="pt")
                    nc.tensor.transpose(
                        pt[:], x2[:, k * 128:(k + 1) * 128], ident[:]
                    )
                    xT = xtpool.tile([128, 128], f32, tag="xT")
                    nc.scalar.copy(out=xT[:], in_=pt[:])
                    nc.tensor.matmul(
                        conv_p[:], lhsT=xT[:], rhs=wm_t[:, k, :],
                        start=(k == 0), stop=(k == NCHUNK - 1),
                    )
                # Epilogue: norms, argmax via is_equal mask, select, relu.
                nc.scalar.activation(
                    out=sq_t[:], in_=conv_p[:],
                    func=mybir.ActivationFunctionType.Square,
                )
                nc.vector.tensor_reduce(
                    out=norm_t[:],
                    in_=sq_t[:].rearrange("p (k d) -> p k d", d=COUT),
                    axis=mybir.AxisListType.X, op=mybir.AluOpType.add,
                )
                nc.vector.tensor_reduce(
                    out=mx_t[:], in_=norm_t[:],
                    axis=mybir.AxisListType.X, op=mybir.AluOpType.max,
                )
                nc.vector.tensor_scalar(
                    out=mask_t[:], in0=norm_t[:], scalar1=mx_t[:],
                    scalar2=None, op0=mybir.AluOpType.is_equal,
                )
                nc.vector.tensor_tensor(
                    out=msel_t[:],
                    in0=conv_p[:].rearrange("p (k d) -> p k d", d=COUT),
                    in1=mask_t[:].unsqueeze(-1).to_broadcast([128, NT, COUT]),
                    op=mybir.AluOpType.mult,
                )
                nc.vector.tensor_reduce(
                    out=o_t[:],
                    in_=msel_t[:].rearrange("p k d -> p d k"),
                    axis=mybir.AxisListType.X, op=mybir.AluOpType.add,
                )
                nc.vector.tensor_scalar_max(o_t[:], o_t[:], 0.0)
                nc.sync.dma_start(out=out_d.ap()[ts(tidx, TPT), :], in_=o_t[:])

            with tc.For_i(0, repeat) as r:
                # 2-stage software pipeline over tiles: even iterations use
                # g0, odd use g1; the prefetch gather of tile p+1 is issued
                # before the compute of tile p.
                gather(i0, g0, 0)
                with tc.For_i(0, NTILES // 2) as h:
                    gather(i1, g1, h * 2 + 1)
                    compute(g0, h * 2)
                    gather(i0, g0, h * 2 + 2)
                    compute(g1, h * 2 + 1)
                # NTILES is odd: last tile (48) was prefetched by the final
                # loop iteration into g0.
                compute(g0, NTILES - 1)

    nc.compile()
    return nc


def make_idx16(idx_vp, ntiles, base):
    """[ntiles*128, E] int32 row indices -> [ntiles*128, NS] wrapped int16.

    Gather list position n = e*128 + p must hold idx[tile*128 + p, e]; the
    ucode reads logical position i from wrapped[i % 16, i // 16], replicated
    across the 8 Q7 cores (16 partitions each). Slot E is a pad row of
    positive indices so the ucode's trailing-negative trim never fires.
    """
    out = np.empty((ntiles, 128, NS), np.int16)
    for t in range(ntiles):
        blk = idx_vp[t * TPT:(t + 1) * TPT]          # [128, E]
        lst = np.full(NIDX, 1, np.int32)             # pad slots -> row base+1
        lst[: E * 128] = (blk.T.astype(np.int32) - base).ravel()  # n = e*128+p
        w = lst.reshape(-1, 16).T                     # [16, NIDX/16]
        out[t] = np.tile(w, (8, 1))
    return out.reshape(ntiles * 128, NS)


def _host_prep(signal, bary_w, bary_idx, kernel):
    """Build per-core input maps. All host-side numpy, not timed."""
    jj = np.arange(NT)
    rot = kernel[:, (jj[:, None] + jj[None, :]) % NT, :, :]  # [i,j,k,c,d]
    wm = np.ascontiguousarray(
        rot.transpose(0, 1, 3, 2, 4).reshape(KC, ND), dtype=np.float32
    )
    sig = np.ascontiguousarray(signal, dtype=np.float32)
    base = 32768
    wv_full = bary_w.reshape(V, E).astype(np.float32)
    idx_full = bary_idx.reshape(V, E).astype(np.int32)
    in_maps = []
    for c in range(NCORES):
        sl = slice(c * VPC, (c + 1) * VPC)
        wv = np.zeros((NTPAD * TPT, E), np.float32)
        wv[:VPC] = wv_full[sl]
        idx = np.zeros((NTPAD * TPT, E), np.int32)
        idx[:VPC] = idx_full[sl]
        in_maps.append({
            "signal": sig,
            "wv": wv,
            "idx16": make_idx16(idx, NTPAD, base),
            "wm": wm,
        })
    return in_maps


def kernel(signal, bary_w, bary_idx, kernel):
    from concourse.bass_utils import run_bass_kernel_spmd

    if "nc" not in _CACHE:
        _CACHE["nc"] = build_program()
    nc = _CACHE["nc"]
    in_maps = _host_prep(signal, bary_w, bary_idx, kernel)
    res = run_bass_kernel_spmd(nc, in_maps, core_ids=list(range(NCORES)))
    out = np.concatenate(
        [res.results[c]["out"][:VPC] for c in range(NCORES)], axis=0
    )
    return out.astype(np.float32)


# revision 3
# speedup vs baseline: 25.8571x; 1.1998x over previous
"""Trainium2 Bass kernel for geodesic convolution (gnn_message_passing).

Computation (per vertex v, with m = (i,j) flattened, NR*NT = 40 slots):
  x[v,m,c]    = sum_t bary_w[v,m,t] * signal[bary_idx[v,m,t], c]
  conv[v,k,d] = sum_{m,c} x[v,m,c] * K[i(m),(j(m)+k)%NT,c,d]
  out[v,:]    = relu(conv[v, argmax_k ||conv[v,k,:]||, :])

Strategy: shard V across 8 cores. The signal stays SBUF-resident in
channel-major transposed form [128 partitions = (h, c), 25024 rows] f32,
split into two row-halves h (int16 gather index limit). The gather runs on
GPSIMD (ap_gather): each Q7 core gathers along its partitions' free dim, so
a position's 64 channels land on 64 partitions at the same free offset.
Positions whose row lies in the other half gather a dummy row and are
masked by zero weights (2-row weight tensor broadcast to 128 partitions by
a tiny PE matmul per 480-column chunk). After the weighted tap-sum, the two
half contributions are merged (SBUF->SBUF DMA partition shift + DVE add)
and re-split by ij-parity into the 128-row lhsT chunks of a 20-chunk f32
matmul accumulation against the pre-rotated kernel matrix. Everything runs
inside tc.For_i hardware loops; tiles are processed in 4 quarters with two
gather buffers so the GPSIMD gather of quarter q+1 overlaps the
PE/DVE/DMA compute of quarter q.
"""

import numpy as np

# Problem constants (hardcoded; kernel.py must be self-contained).
V, NR, NT, CIN, COUT = 50000, 5, 8, 64, 64
NCORES = 8
VPC = V // NCORES            # 6250 vertices per core
TPT = 128                    # vertices per tile
NTILES = -(-VPC // TPT)      # 49
VPAD = NTILES * TPT          # 6272
M = NR * NT                  # 40 (i,j) slots
NPOS = M * TPT * 3           # 15360 gather positions per tile
HALF = 25024                 # rows per signal half ((h, c) partition split)
NQ = 4                       # quarters per tile
MQ = M // NQ                 # 10 m-slots per quarter
QPOS = NPOS // NQ            # 3840 positions per quarter
CCH = 480                    # weight-broadcast chunk columns
NCH = QPOS // CCH            # 8 chunks per quarter
KD = NT * COUT               # 512 output cols (k,d)
NPAIR = M // 2               # 20 matmul chunks (ij-parity pairs)

_CACHE = {}


def build_program(repeat=1):
    """Build the Bacc program for one SPMD core. Returns compiled nc.

    repeat is a For_i loop bound around the whole tile loop; static program
    size is identical for any repeat, so the wall-clock slope between
    repeat values measures pure per-pass hardware execution time.
    """
    import concourse.bass as bass
    import concourse.mybir as mybir
    import concourse.tile as tile
    from concourse import bacc
    from concourse.bass import ts

    f32 = mybir.dt.float32
    i16 = mybir.dt.int16

    nc = bacc.Bacc(
        "TRN2",
        target_bir_lowering=False,
        debug=False,
        enable_asserts=False,
        num_devices=NCORES,
    )
    from concourse.bass import ds

    sig_d = nc.dram_tensor("sig2", [128, HALF], f32, kind="ExternalInput")
    idx_d = nc.dram_tensor("idx16", [NTILES * TPT, NPOS // 16], i16,
                           kind="ExternalInput")
    wv_d = nc.dram_tensor("wv2", [NTILES * 2, NPOS], f32,
                          kind="ExternalInput")
    sel_d = nc.dram_tensor("sel", [2, 128], f32, kind="ExternalInput")
    wm_d = nc.dram_tensor("wm2", [128, NPAIR * KD], f32, kind="ExternalInput")
    out_d = nc.dram_tensor("out", [VPAD, COUT], f32, kind="ExternalOutput")

    with tile.TileContext(nc) as tc:
        with (
            tc.tile_pool(name="const", bufs=1) as cpool,
            tc.tile_pool(name="io", bufs=1) as iopool,
            tc.tile_pool(name="g", bufs=1) as gpool,
            tc.tile_pool(name="lt", bufs=1) as ltpool,
            tc.tile_pool(name="epi", bufs=1) as epool,
            tc.tile_pool(name="psA", bufs=1, space="PSUM") as psA,
            tc.tile_pool(name="psB", bufs=2, space="PSUM") as psB,
        ):
            sig_t = cpool.tile([128, HALF], f32)
            nc.sync.dma_start(out=sig_t[:], in_=sig_d.ap())
            wm_t = cpool.tile([128, NPAIR, KD], f32)
            nc.sync.dma_start(
                out=wm_t[:], in_=wm_d.ap().rearrange("p (m n) -> p m n",
                                                     m=NPAIR))

            sel_t = cpool.tile([2, 128], f32)
            nc.sync.dma_start(out=sel_t[:], in_=sel_d.ap())
            i_t = iopool.tile([128, NPOS // 16], i16, tag="i")
            wv_t = iopool.tile([2, QPOS], f32, tag="wv")
            g0 = gpool.tile([128, QPOS], f32, tag="g0")
            g1 = gpool.tile([128, QPOS], f32, tag="g1")
            lt_t = ltpool.tile([128, NPAIR // NQ, 128], f32)
            sq_t = epool.tile([128, KD], f32, tag="sq")
            norm_t = epool.tile([128, NT], f32, tag="norm")
            mx_t = epool.tile([128, 1], f32, tag="mx")
            mask_t = epool.tile([128, NT], f32, tag="mask")
            msel_t = epool.tile([128, NT, COUT], f32, tag="msel")
            o_t = epool.tile([128, COUT], f32, tag="o")

            XT2 = QPOS // 3          # 1280: tap-summed cols per quarter
            gbufs = [g0, g1]

            def gather(q):
                g_t = gbufs[q % 2]
                nc.gpsimd.ap_gather(
                    out_ap=g_t[:].unsqueeze(-1),
                    in_ap=sig_t[:].unsqueeze(-1),
                    idxs_ap=i_t[:, q * (QPOS // 16):(q + 1) * (QPOS // 16)],
                    channels=128, num_elems=HALF, d=1, num_idxs=QPOS,
                )

            def compute(q, t, conv_p):
                g_t = gbufs[q % 2]
                # stage the (lo-masked, hi-masked) weight rows, then expand
                # them to the matching partition halves with tiny PE matmuls
                # (sel[h',p] = (p div 64 == h')), multiplying in-place per
                # 480-column PSUM chunk.
                qs = slice(q * QPOS, (q + 1) * QPOS)
                nc.sync.dma_start(out=wv_t[:], in_=wv_d.ap()[ds(t * 2, 2), qs])
                for ch in range(NCH):
                    pw = psB.tile([128, CCH], f32, tag="pw")
                    nc.tensor.matmul(
                        pw[:], lhsT=sel_t[:],
                        rhs=wv_t[:, ch * CCH:(ch + 1) * CCH],
                        start=True, stop=True,
                    )
                    nc.vector.tensor_tensor(
                        out=g_t[:, ch * CCH:(ch + 1) * CCH],
                        in0=g_t[:, ch * CCH:(ch + 1) * CCH],
                        in1=pw[:],
                        op=mybir.AluOpType.mult,
                    )
                # tap-sum into g[:, :XT2] (in-place strided: write n < read 3n)
                nc.vector.tensor_reduce(
                    out=g_t[:, :XT2],
                    in_=g_t[:].rearrange("p (n t) -> p n t", t=3),
                    axis=mybir.AxisListType.X, op=mybir.AluOpType.add,
                )
                # h-merge: shift hi partitions down, then add
                nc.sync.dma_start(out=g_t[0:64, XT2:2 * XT2],
                                  in_=g_t[64:128, 0:XT2])
                nc.vector.tensor_tensor(
                    out=g_t[0:64, 2 * XT2:3 * XT2],
                    in0=g_t[0:64, 0:XT2],
                    in1=g_t[0:64, XT2:2 * XT2],
                    op=mybir.AluOpType.add,
                )
                # o-split: build lhsT chunks [(o,c), k', v] via 2 SBUF DMAs
                xts = g_t[0:64, 2 * XT2:3 * XT2].rearrange(
                    "p (a v) -> p a v", v=128)  # a = m_local in [0,10)
                nc.sync.dma_start(
                    out=lt_t[0:64, :, :],
                    in_=xts[:, 0::2, :],
                )
                nc.sync.dma_start(
                    out=lt_t[64:128, :, :],
                    in_=xts[:, 1::2, :],
                )
                for kp in range(NPAIR // NQ):
                    mp = q * (NPAIR // NQ) + kp
                    nc.tensor.matmul(
                        conv_p[:], lhsT=lt_t[:, kp, :], rhs=wm_t[:, mp, :],
                        start=(mp == 0), stop=(mp == NPAIR - 1),
                    )

            with tc.For_i(0, repeat) as r:
                with tc.For_i(0, NTILES) as t:
                    nc.sync.dma_start(out=i_t[:],
                                      in_=idx_d.ap()[ts(t, TPT), :])
                    conv_p = psA.tile([128, KD], f32, tag="conv")
                    gather(0)
                    for q in range(NQ):
                        if q + 1 < NQ:
                            gather(q + 1)
                        compute(q, t, conv_p)
                    # Epilogue: norms, argmax via is_equal mask, select, relu.
                    nc.scalar.activation(
                        out=sq_t[:], in_=conv_p[:],
                        func=mybir.ActivationFunctionType.Square,
                    )
                    nc.vector.tensor_reduce(
                        out=norm_t[:],
                        in_=sq_t[:].rearrange("p (k d) -> p k d", d=COUT),
                        axis=mybir.AxisListType.X, op=mybir.AluOpType.add,
                    )
                    nc.vector.tensor_reduce(
                        out=mx_t[:], in_=norm_t[:],
                        axis=mybir.AxisListType.X, op=mybir.AluOpType.max,
                    )
                    nc.vector.tensor_scalar(
                        out=mask_t[:], in0=norm_t[:], scalar1=mx_t[:],
                        scalar2=None, op0=mybir.AluOpType.is_equal,
                    )
                    nc.vector.tensor_tensor(
                        out=msel_t[:],
                        in0=conv_p[:].rearrange("p (k d) -> p k d", d=COUT),
                        in1=mask_t[:].unsqueeze(-1).to_broadcast(
                            [128, NT, COUT]),
                        op=mybir.AluOpType.mult,
                    )
                    nc.vector.tensor_reduce(
                        out=o_t[:],
                        in_=msel_t[:].rearrange("p k d -> p d k"),
                        axis=mybir.AxisListType.X, op=mybir.AluOpType.add,
                    )
                    nc.vector.tensor_scalar_max(o_t[:], o_t[:], 0.0)
                    nc.sync.dma_start(out=out_d.ap()[ts(t, TPT), :],
                                      in_=o_t[:])

    nc.compile()
    return nc


def _host_prep(signal, bary_w, bary_idx, kernel):
    """Build per-core input maps. All host-side numpy, not timed."""
    sig = np.asarray(signal, np.float32)
    sig2 = np.zeros((128, HALF), np.float32)
    sig2[:64, :HALF] = sig[:HALF].T
    sig2[64:, :V - HALF] = sig[HALF:].T

    jj = np.arange(NT)
    rot = kernel[:, (jj[:, None] + jj[None, :]) % NT, :, :]  # [i,j,k,c,d]
    wm = np.ascontiguousarray(
        rot.transpose(0, 1, 3, 2, 4).reshape(M * CIN, KD), np.float32
    )  # row (m, c)
    wm2 = np.ascontiguousarray(
        wm.reshape(NPAIR, 2, CIN, KD).transpose(1, 2, 0, 3)
        .reshape(128, NPAIR * KD), np.float32
    )  # row (o*64 + c), cols (m_pair, kd)

    idx_full = bary_idx.reshape(V, M, 3).astype(np.int32)
    wv_full = bary_w.reshape(V, M, 3).astype(np.float32)

    in_maps = []
    for c in range(NCORES):
        sl = slice(c * VPC, (c + 1) * VPC)
        idx_c = np.zeros((VPAD, M, 3), np.int32)
        idx_c[:VPC] = idx_full[sl]
        wv_c = np.zeros((VPAD, M, 3), np.float32)
        wv_c[:VPC] = wv_full[sl]

        # position order within a tile: n = m*384 + v*3 + t
        r = (idx_c.reshape(NTILES, TPT, M, 3)
             .transpose(0, 2, 1, 3).reshape(NTILES, NPOS))
        w = (wv_c.reshape(NTILES, TPT, M, 3)
             .transpose(0, 2, 1, 3).reshape(NTILES, NPOS))

        lo = np.where(r < HALF, r, 0).astype(np.int16)
        hi = np.where(r >= HALF, r - HALF, 0).astype(np.int16)
        idx16 = np.zeros((NTILES, 128, NPOS // 16), np.int16)
        # per quarter, wrapped in 16 within each quarter's index range
        for q in range(NQ):
            s = slice(q * QPOS, (q + 1) * QPOS)
            cs = slice(q * (QPOS // 16), (q + 1) * (QPOS // 16))
            wlo = lo[:, s].reshape(NTILES, QPOS // 16, 16).transpose(0, 2, 1)
            whi = hi[:, s].reshape(NTILES, QPOS // 16, 16).transpose(0, 2, 1)
            idx16[:, :64, cs] = np.tile(wlo, (1, 4, 1))
            idx16[:, 64:, cs] = np.tile(whi, (1, 4, 1))

        wv2 = np.zeros((NTILES, 2, NPOS), np.float32)
        wv2[:, 0] = w * (r < HALF)
        wv2[:, 1] = w * (r >= HALF)
        sel = np.zeros((2, 128), np.float32)
        sel[0, :64] = 1.0
        sel[1, 64:] = 1.0

        in_maps.append({
            "sig2": sig2,
            "idx16": idx16.reshape(NTILES * TPT, NPOS // 16),
            "wv2": wv2.reshape(NTILES * 2, NPOS),
            "sel": sel,
            "wm2": wm2,
        })
    return in_maps


def kernel(signal, bary_w, bary_idx, kernel):
    from concourse.bass_utils import run_bass_kernel_spmd

    if "nc" not in _CACHE:
        _CACHE["nc"] = build_program()
    nc = _CACHE["nc"]
    in_maps = _host_prep(signal, bary_w, bary_idx, kernel)
    res = run_bass_kernel_spmd(nc, in_maps, core_ids=list(range(NCORES)))
    out = np.concatenate(
        [res.results[c]["out"][:VPC] for c in range(NCORES)], axis=0
    )
    return out.astype(np.float32)


# revision 4
# speedup vs baseline: 28.2356x; 1.0920x over previous
"""Trainium2 Bass kernel for geodesic convolution (gnn_message_passing).

Computation (per vertex v, with m = (i,j) flattened, NR*NT = 40 slots):
  x[v,m,c]    = sum_t bary_w[v,m,t] * signal[bary_idx[v,m,t], c]
  conv[v,k,d] = sum_{m,c} x[v,m,c] * K[i(m),(j(m)+k)%NT,c,d]
  out[v,:]    = relu(conv[v, argmax_k ||conv[v,k,:]||, :])

Strategy: shard V across 8 cores. The signal stays SBUF-resident in
channel-major transposed form [128 partitions = (h, c), 25024 rows] f32,
split into two row-halves h (int16 gather index limit). The gather runs on
GPSIMD (ap_gather): each Q7 core gathers along its partitions' free dim, so
a position's 64 channels land on 64 partitions at the same free offset.
Positions whose row lies in the other half gather a dummy row and are
masked by zero weights (2-row weight tensor broadcast to 128 partitions by
a tiny PE matmul per 480-column chunk). After the weighted tap-sum, the two
half contributions are merged (SBUF->SBUF DMA partition shift + DVE add)
and re-split by ij-parity into the 128-row lhsT chunks of a 20-chunk f32
matmul accumulation against the pre-rotated kernel matrix. Everything runs
inside tc.For_i hardware loops; tiles are processed in 4 quarters with two
gather buffers so the GPSIMD gather of quarter q+1 overlaps the
PE/DVE/DMA compute of quarter q.
"""

import numpy as np

# Problem constants (hardcoded; kernel.py must be self-contained).
V, NR, NT, CIN, COUT = 50000, 5, 8, 64, 64
NCORES = 8
VPC = V // NCORES            # 6250 vertices per core
TPT = 128                    # vertices per tile
NTILES = -(-VPC // TPT)      # 49
VPAD = NTILES * TPT          # 6272
M = NR * NT                  # 40 (i,j) slots
NPOS = M * TPT * 3           # 15360 gather positions per tile
HALF = 25024                 # rows per signal half ((h, c) partition split)
NQ = 4                       # quarters per tile
MQ = M // NQ                 # 10 m-slots per quarter
QPOS = NPOS // NQ            # 3840 positions per quarter
CCH = 480                    # weight-broadcast chunk columns
NCH = QPOS // CCH            # 8 chunks per quarter
KD = NT * COUT               # 512 output cols (k,d)
NPAIR = M // 2               # 20 matmul chunks (ij-parity pairs)

_CACHE = {}


def build_program(repeat=1):
    """Build the Bacc program for one SPMD core. Returns compiled nc.

    repeat is a For_i loop bound around the whole tile loop; static program
    size is identical for any repeat, so the wall-clock slope between
    repeat values measures pure per-pass hardware execution time.
    """
    import concourse.bass as bass
    import concourse.mybir as mybir
    import concourse.tile as tile
    from concourse import bacc
    from concourse.bass import ts

    f32 = mybir.dt.float32
    i16 = mybir.dt.int16

    nc = bacc.Bacc(
        "TRN2",
        target_bir_lowering=False,
        debug=False,
        enable_asserts=False,
        num_devices=NCORES,
    )
    from concourse.bass import ds

    sig_d = nc.dram_tensor("sig2", [128, HALF], f32, kind="ExternalInput")
    idx_d = nc.dram_tensor("idx16", [NTILES * TPT, NPOS // 16], i16,
                           kind="ExternalInput")
    wv_d = nc.dram_tensor("wv2", [NTILES * 2, NPOS], f32,
                          kind="ExternalInput")
    sel_d = nc.dram_tensor("sel", [2, 128], f32, kind="ExternalInput")
    wm_d = nc.dram_tensor("wm2", [128, NPAIR * KD], f32, kind="ExternalInput")
    out_d = nc.dram_tensor("out", [VPAD, COUT], f32, kind="ExternalOutput")

    with tile.TileContext(nc) as tc:
        with (
            tc.tile_pool(name="const", bufs=1) as cpool,
            tc.tile_pool(name="io", bufs=1) as iopool,
            tc.tile_pool(name="g", bufs=1) as gpool,
            tc.tile_pool(name="lt", bufs=1) as ltpool,
            tc.tile_pool(name="epi", bufs=1) as epool,
            tc.tile_pool(name="psA", bufs=2, space="PSUM") as psA,
            tc.tile_pool(name="psB", bufs=2, space="PSUM") as psB,
        ):
            sig_t = cpool.tile([128, HALF], f32)
            nc.sync.dma_start(out=sig_t[:], in_=sig_d.ap())
            wm_t = cpool.tile([128, NPAIR, KD], f32)
            nc.sync.dma_start(
                out=wm_t[:], in_=wm_d.ap().rearrange("p (m n) -> p m n",
                                                     m=NPAIR))

            sel_t = cpool.tile([2, 128], f32)
            nc.sync.dma_start(out=sel_t[:], in_=sel_d.ap())
            i_a = iopool.tile([128, NPOS // 16], i16, tag="ia")
            i_b = iopool.tile([128, NPOS // 16], i16, tag="ib")
            wv_t = iopool.tile([2, QPOS], f32, tag="wv")
            g0 = gpool.tile([128, QPOS], f32, tag="g0")
            g1 = gpool.tile([128, QPOS], f32, tag="g1")
            lt_a = ltpool.tile([128, NPAIR // NQ, 128], f32, tag="la")
            lt_b = ltpool.tile([128, NPAIR // NQ, 128], f32, tag="lb")
            sq_t = epool.tile([128, KD], f32, tag="sq")
            norm_t = epool.tile([128, NT], f32, tag="norm")
            mx_t = epool.tile([128, 1], f32, tag="mx")
            mask_t = epool.tile([128, NT], f32, tag="mask")
            msel_t = epool.tile([128, NT, COUT], f32, tag="msel")
            o_t = epool.tile([128, COUT], f32, tag="o")

            XT2 = QPOS // 3          # 1280: tap-summed cols per quarter
            gbufs = [g0, g1]

            def gather(gq, i_t):
                q = gq % NQ
                g_t = gbufs[gq % 2]
                nc.gpsimd.ap_gather(
                    out_ap=g_t[:].unsqueeze(-1),
                    in_ap=sig_t[:].unsqueeze(-1),
                    idxs_ap=i_t[:, q * (QPOS // 16):(q + 1) * (QPOS // 16)],
                    channels=128, num_elems=HALF, d=1, num_idxs=QPOS,
                )

            def compute(gq, t, conv_p, lt_t):
                q = gq % NQ
                g_t = gbufs[gq % 2]
                # stage the (lo-masked, hi-masked) weight rows, then expand
                # them to the matching partition halves with tiny PE matmuls
                # (sel[h',p] = (p div 64 == h')), multiplying in-place per
                # 480-column PSUM chunk.
                qs = slice(q * QPOS, (q + 1) * QPOS)
                nc.sync.dma_start(out=wv_t[:], in_=wv_d.ap()[ds(t * 2, 2), qs])
                for ch in range(NCH):
                    pw = psB.tile([128, CCH], f32, tag="pw")
                    nc.tensor.matmul(
                        pw[:], lhsT=sel_t[:],
                        rhs=wv_t[:, ch * CCH:(ch + 1) * CCH],
                        start=True, stop=True,
                    )
                    nc.vector.tensor_tensor(
                        out=g_t[:, ch * CCH:(ch + 1) * CCH],
                        in0=g_t[:, ch * CCH:(ch + 1) * CCH],
                        in1=pw[:],
                        op=mybir.AluOpType.mult,
                    )
                # tap-sum into g[:, :XT2] (in-place strided: write n < read 3n)
                nc.vector.tensor_reduce(
                    out=g_t[:, :XT2],
                    in_=g_t[:].rearrange("p (n t) -> p n t", t=3),
                    axis=mybir.AxisListType.X, op=mybir.AluOpType.add,
                )
                # h-merge: shift hi partitions down, then add
                nc.sync.dma_start(out=g_t[0:64, XT2:2 * XT2],
                                  in_=g_t[64:128, 0:XT2])
                nc.vector.tensor_tensor(
                    out=g_t[0:64, 2 * XT2:3 * XT2],
                    in0=g_t[0:64, 0:XT2],
                    in1=g_t[0:64, XT2:2 * XT2],
                    op=mybir.AluOpType.add,
                )
                # o-split: build lhsT chunks [(o,c), k', v] via 2 SBUF DMAs
                xts = g_t[0:64, 2 * XT2:3 * XT2].rearrange(
                    "p (a v) -> p a v", v=128)  # a = m_local in [0,10)
                nc.sync.dma_start(
                    out=lt_t[0:64, :, :],
                    in_=xts[:, 0::2, :],
                )
                nc.sync.dma_start(
                    out=lt_t[64:128, :, :],
                    in_=xts[:, 1::2, :],
                )
                for kp in range(NPAIR // NQ):
                    mp = q * (NPAIR // NQ) + kp
                    nc.tensor.matmul(
                        conv_p[:], lhsT=lt_t[:, kp, :], rhs=wm_t[:, mp, :],
                        start=(mp == 0), stop=(mp == NPAIR - 1),
                    )

            def epilogue(conv_p, t):
                # norms, argmax via is_equal mask, select, relu, store.
                nc.scalar.activation(
                    out=sq_t[:], in_=conv_p[:],
                    func=mybir.ActivationFunctionType.Square,
                )
                nc.vector.tensor_reduce(
                    out=norm_t[:],
                    in_=sq_t[:].rearrange("p (k d) -> p k d", d=COUT),
                    axis=mybir.AxisListType.X, op=mybir.AluOpType.add,
                )
                nc.vector.tensor_reduce(
                    out=mx_t[:], in_=norm_t[:],
                    axis=mybir.AxisListType.X, op=mybir.AluOpType.max,
                )
                nc.vector.tensor_scalar(
                    out=mask_t[:], in0=norm_t[:], scalar1=mx_t[:],
                    scalar2=None, op0=mybir.AluOpType.is_equal,
                )
                nc.vector.tensor_tensor(
                    out=msel_t[:],
                    in0=conv_p[:].rearrange("p (k d) -> p k d", d=COUT),
                    in1=mask_t[:].unsqueeze(-1).to_broadcast(
                        [128, NT, COUT]),
                    op=mybir.AluOpType.mult,
                )
                nc.vector.tensor_reduce(
                    out=o_t[:],
                    in_=msel_t[:].rearrange("p k d -> p d k"),
                    axis=mybir.AxisListType.X, op=mybir.AluOpType.add,
                )
                nc.vector.tensor_scalar_max(o_t[:], o_t[:], 0.0)
                nc.sync.dma_start(out=out_d.ap()[ts(t, TPT), :], in_=o_t[:])

            def tile_pair(te, to):
                # 8 quarters across two tiles; the gather of quarter gq+1 is
                # issued before the compute of quarter gq so GPSIMD never
                # idles, including across the tile boundary.
                nc.sync.dma_start(out=i_a[:], in_=idx_d.ap()[ts(te, TPT), :])
                nc.sync.dma_start(out=i_b[:], in_=idx_d.ap()[ts(to, TPT), :])
                conv_e = psA.tile([128, KD], f32, tag="conv")
                conv_o = psA.tile([128, KD], f32, tag="conv")
                parts = [(i_a, te, conv_e, lt_a), (i_b, to, conv_o, lt_b)]
                gather(0, i_a)
                for gq in range(1, 2 * NQ):
                    gather(gq, parts[gq // NQ][0])
                    i_, t_, c_, l_ = parts[(gq - 1) // NQ]
                    compute(gq - 1, t_, c_, l_)
                    if gq - 1 == NQ - 1:
                        epilogue(conv_e, te)
                compute(2 * NQ - 1, to, conv_o, lt_b)
                epilogue(conv_o, to)

            with tc.For_i(0, repeat) as r:
                with tc.For_i(0, NTILES // 2) as hh:
                    tile_pair(hh * 2, hh * 2 + 1)
                # tail tile (NTILES is odd)
                tl = NTILES - 1
                nc.sync.dma_start(out=i_a[:], in_=idx_d.ap()[ts(tl, TPT), :])
                conv_t = psA.tile([128, KD], f32, tag="conv")
                gather(0, i_a)
                for q in range(NQ):
                    if q + 1 < NQ:
                        gather(q + 1, i_a)
                    compute(q, tl, conv_t, lt_a)
                epilogue(conv_t, tl)

    nc.compile()
    return nc


def _host_prep(signal, bary_w, bary_idx, kernel):
    """Build per-core input maps. All host-side numpy, not timed."""
    sig = np.asarray(signal, np.float32)
    sig2 = np.zeros((128, HALF), np.float32)
    sig2[:64, :HALF] = sig[:HALF].T
    sig2[64:, :V - HALF] = sig[HALF:].T

    jj = np.arange(NT)
    rot = kernel[:, (jj[:, None] + jj[None, :]) % NT, :, :]  # [i,j,k,c,d]
    wm = np.ascontiguousarray(
        rot.transpose(0, 1, 3, 2, 4).reshape(M * CIN, KD), np.float32
    )  # row (m, c)
    wm2 = np.ascontiguousarray(
        wm.reshape(NPAIR, 2, CIN, KD).transpose(1, 2, 0, 3)
        .reshape(128, NPAIR * KD), np.float32
    )  # row (o*64 + c), cols (m_pair, kd)

    idx_full = bary_idx.reshape(V, M, 3).astype(np.int32)
    wv_full = bary_w.reshape(V, M, 3).astype(np.float32)

    in_maps = []
    for c in range(NCORES):
        sl = slice(c * VPC, (c + 1) * VPC)
        idx_c = np.zeros((VPAD, M, 3), np.int32)
        idx_c[:VPC] = idx_full[sl]
        wv_c = np.zeros((VPAD, M, 3), np.float32)
        wv_c[:VPC] = wv_full[sl]

        # position order within a tile: n = m*384 + v*3 + t
        r = (idx_c.reshape(NTILES, TPT, M, 3)
             .transpose(0, 2, 1, 3).reshape(NTILES, NPOS))
        w = (wv_c.reshape(NTILES, TPT, M, 3)
             .transpose(0, 2, 1, 3).reshape(NTILES, NPOS))

        lo = np.where(r < HALF, r, 0).astype(np.int16)
        hi = np.where(r >= HALF, r - HALF, 0).astype(np.int16)
        idx16 = np.zeros((NTILES, 128, NPOS // 16), np.int16)
        # per quarter, wrapped in 16 within each quarter's index range
        for q in range(NQ):
            s = slice(q * QPOS, (q + 1) * QPOS)
            cs = slice(q * (QPOS // 16), (q + 1) * (QPOS // 16))
            wlo = lo[:, s].reshape(NTILES, QPOS // 16, 16).transpose(0, 2, 1)
            whi = hi[:, s].reshape(NTILES, QPOS // 16, 16).transpose(0, 2, 1)
            idx16[:, :64, cs] = np.tile(wlo, (1, 4, 1))
            idx16[:, 64:, cs] = np.tile(whi, (1, 4, 1))

        wv2 = np.zeros((NTILES, 2, NPOS), np.float32)
        wv2[:, 0] = w * (r < HALF)
        wv2[:, 1] = w * (r >= HALF)
        sel = np.zeros((2, 128), np.float32)
        sel[0, :64] = 1.0
        sel[1, 64:] = 1.0

        in_maps.append({
            "sig2": sig2,
            "idx16": idx16.reshape(NTILES * TPT, NPOS // 16),
            "wv2": wv2.reshape(NTILES * 2, NPOS),
            "sel": sel,
            "wm2": wm2,
        })
    return in_maps


def kernel(signal, bary_w, bary_idx, kernel):
    from concourse.bass_utils import run_bass_kernel_spmd

    if "nc" not in _CACHE:
        _CACHE["nc"] = build_program()
    nc = _CACHE["nc"]
    in_maps = _host_prep(signal, bary_w, bary_idx, kernel)
    res = run_bass_kernel_spmd(nc, in_maps, core_ids=list(range(NCORES)))
    out = np.concatenate(
        [res.results[c]["out"][:VPC] for c in range(NCORES)], axis=0
    )
    return out.astype(np.float32)
